# revision 15
# baseline (speedup 1.0000x reference)
"""Trainium2 Bass kernel for nn_BLoss: loss = mean_i(max(0, sum_j B[i,j] - 1)).

Data-parallel over 8 NeuronCores: each core streams a [1024, 16384] row shard
through SBUF in [128, 8192] chunks (32 KiB descriptors measured at ~415 GB/s,
gap-free). Row-chunk sums are computed on two engines in parallel — VectorE
tensor_reduce and ScalarE activation-Copy with accum_out — so compute never
lags the stream. The final row tile's second half streams in descending chunks
so the post-stream backlog is ~1.5 us. Hinge max(rowsum-1, 0) on VectorE; the
128 per-partition partials are collapsed with a ones-matmul on TensorE so the
output DMA is a single 4-byte descriptor (a [128,1] output pays ~7 us in
straggling DMA completion receipts). Host sums 8 per-core scalars and divides
by the global batch.
"""

import numpy as np
from contextlib import ExitStack

import concourse.bass as bass
import concourse.tile as tile
from concourse import bacc, mybir
from concourse.bass_utils import run_bass_kernel_spmd

N_CORES = 8
ROWS, COLS = 8192, 16384
SHARD_ROWS = ROWS // N_CORES  # 1024
P = 128                       # SBUF partitions
N_RT = SHARD_ROWS // P        # 8 row tiles per core
CHUNK = 8192
PENALTY_B = 1.0

# Tile 7 = one full chunk + descending tail so the post-stream backlog is
# small. DVE_COLS gives VectorE's share of each tail-region chunk, tuned so
# VectorE (1.04 ns/col) and ScalarE (0.85 ns/col + fixed accum-read) both
# drain right as the last bytes land.
TAIL_CHUNKS = [4096, 2048, 1536, 512]
TAIL_DVE_COLS = {14: 4600, 15: 2200, 16: 1050, 17: 800, 18: 465}
N_FULL_CHUNKS = (N_RT - 1) * 2 + 1    # 15 chunks of 8192
N_PARTS = N_FULL_CHUNKS + len(TAIL_CHUNKS)

_PROGRAM = None


def _build_program() -> bass.Bass:
    nc = bacc.Bacc("TRN2", target_bir_lowering=False, debug=False)
    B = nc.declare_dram_parameter(
        "B", [SHARD_ROWS, COLS], mybir.dt.float32, isOutput=False
    )
    out = nc.declare_dram_parameter("out", [1, 1], mybir.dt.float32, isOutput=True)

    with ExitStack() as ctx:
        tc = ctx.enter_context(tile.TileContext(nc))
        data = ctx.enter_context(tc.tile_pool(name="data", bufs=4))
        stats = ctx.enter_context(tc.tile_pool(name="stats", bufs=1))
        psum = ctx.enter_context(tc.tile_pool(name="psum", bufs=1, space="PSUM"))

        # Chunk i's DVE partial lands in col 2i, its ACT partial in col 2i+1,
        # so each row tile's partials are contiguous and need no combine op.
        sums = stats.tile([P, 2 * N_PARTS], mybir.dt.float32)
        dummy = stats.tile([P, CHUNK - TAIL_DVE_COLS[14]], mybir.dt.float32)
        ones = stats.tile([P, 1], mybir.dt.float32)
        nc.vector.memset(ones[:], 1.0)

        # (row_tile, col_offset, width) per chunk, linear order.
        chunks = []
        for r in range(N_RT - 1):
            chunks.append((r, 0, CHUNK))
            chunks.append((r, CHUNK, CHUNK))
        chunks.append((N_RT - 1, 0, CHUNK))
        col = CHUNK
        for w in TAIL_CHUNKS:
            chunks.append((N_RT - 1, col, w))
            col += w

        for i, (r, c0, w) in enumerate(chunks):
            t = data.tile([P, w], mybir.dt.float32, tag="t")
            nc.sync.dma_start(t[:], B[r * P : (r + 1) * P, c0 : c0 + w])
            dve_cols = TAIL_DVE_COLS.get(i, (w * 5) // 8)
            nc.vector.reduce_sum(
                sums[:, 2 * i : 2 * i + 1], t[:, :dve_cols], axis=mybir.AxisListType.X
            )
            nc.scalar.activation(
                dummy[:, : w - dve_cols],
                t[:, dve_cols:w],
                mybir.ActivationFunctionType.Copy,
                accum_out=sums[:, 2 * i + 1 : 2 * i + 2],
            )

        # Per-row-tile sums: full tiles have 4 contiguous partials, tile 7
        # has 8 (one full chunk + three tail chunks).
        n_t7 = 2 * (1 + len(TAIL_CHUNKS))
        rowsums = stats.tile([P, N_RT], mybir.dt.float32)
        nc.vector.reduce_sum(
            rowsums[:, : N_RT - 1],
            sums[:, : 2 * N_PARTS - n_t7].rearrange("p (r c) -> p r c", c=4),
            axis=mybir.AxisListType.X,
        )
        nc.vector.reduce_sum(
            rowsums[:, N_RT - 1 : N_RT],
            sums[:, 2 * N_PARTS - n_t7 :],
            axis=mybir.AxisListType.X,
        )

        # hinge = max(rowsum - 1, 0); per-partition sum; partition-collapse.
        hinges = stats.tile([P, N_RT], mybir.dt.float32)
        nc.vector.tensor_scalar(
            hinges[:],
            rowsums[:],
            -1.0,
            0.0,
            op0=mybir.AluOpType.add,
            op1=mybir.AluOpType.max,
        )
        hsum = stats.tile([P, 1], mybir.dt.float32)
        nc.vector.reduce_sum(hsum[:], hinges[:], axis=mybir.AxisListType.X)

        acc = psum.tile([1, 1], mybir.dt.float32)
        nc.tensor.matmul(acc[:], ones[:], hsum[:], start=True, stop=True)
        res = stats.tile([1, 1], mybir.dt.float32)
        nc.scalar.copy(res[:], acc[:])
        nc.sync.dma_start(out[:], res[:])

    nc.compile()
    return nc


def _run(B: np.ndarray, trace: bool = False):
    global _PROGRAM
    if _PROGRAM is None:
        _PROGRAM = _build_program()
    in_maps = [
        {"B": B[i * SHARD_ROWS : (i + 1) * SHARD_ROWS]} for i in range(N_CORES)
    ]
    res = run_bass_kernel_spmd(_PROGRAM, in_maps, list(range(N_CORES)), trace=trace)
    total = float(sum(np.float64(r["out"][0, 0]) for r in res.results))
    value = np.asarray(np.float32(PENALTY_B * total / ROWS))
    return value, res


def kernel(B: np.ndarray) -> np.ndarray:
    B = np.ascontiguousarray(np.asarray(B, dtype=np.float32))
    assert B.shape == (ROWS, COLS), B.shape
    value, _ = _run(B, trace=False)
    return value


# revision 17
# speedup vs baseline: 1.1393x; 1.1393x over previous
"""Trainium2 Bass kernel for nn_BLoss: loss = mean_i(max(0, sum_j B[i,j] - 1)).

Data-parallel over 8 NeuronCores: each core streams a [1024, 16384] row shard
through SBUF in [128, 8192] chunks (32 KiB descriptors measured at ~415 GB/s,
gap-free). Row-chunk sums are computed on two engines in parallel — VectorE
tensor_reduce and ScalarE activation-Copy with accum_out — so compute never
lags the stream. The final row tile's second half streams in descending chunks
so the post-stream backlog is ~1.5 us. Hinge max(rowsum-1, 0) on VectorE; the
128 per-partition partials are collapsed with a ones-matmul on TensorE so the
output DMA is a single 4-byte descriptor (a [128,1] output pays ~7 us in
straggling DMA completion receipts). Host sums 8 per-core scalars and divides
by the global batch.
"""

import numpy as np
from contextlib import ExitStack

import concourse.bass as bass
import concourse.tile as tile
from concourse import bacc, mybir
from concourse.bass_utils import run_bass_kernel_spmd

N_CORES = 8
ROWS, COLS = 8192, 16384
SHARD_ROWS = ROWS // N_CORES  # 1024
P = 128                       # SBUF partitions
N_RT = SHARD_ROWS // P        # 8 row tiles per core
CHUNK = 8192
PENALTY_B = 1.0

# Tile 7 = one full chunk + descending tail so the post-stream backlog is
# small. DVE_COLS gives VectorE's share of a chunk; the rest goes to ScalarE.
# The last chunk's split balances VectorE (1.04 ns/col) against ScalarE
# (0.85 ns/col + fixed accum-read) so both drain as the last bytes land.
TAIL_CHUNKS = [4096, 2048, 2048]
TAIL_DVE_COLS = {17: 1152}
N_FULL_CHUNKS = (N_RT - 1) * 2 + 1    # 15 chunks of 8192
N_PARTS = N_FULL_CHUNKS + len(TAIL_CHUNKS)

_PROGRAM = None


def _build_program() -> bass.Bass:
    nc = bacc.Bacc("TRN2", target_bir_lowering=False, debug=False)
    B = nc.declare_dram_parameter(
        "B", [SHARD_ROWS, COLS], mybir.dt.float32, isOutput=False
    )
    out = nc.declare_dram_parameter("out", [1, 1], mybir.dt.float32, isOutput=True)

    with ExitStack() as ctx:
        tc = ctx.enter_context(tile.TileContext(nc))
        data = ctx.enter_context(tc.tile_pool(name="data", bufs=4))
        stats = ctx.enter_context(tc.tile_pool(name="stats", bufs=1))
        psum = ctx.enter_context(tc.tile_pool(name="psum", bufs=1, space="PSUM"))

        # Chunk i's DVE partial lands in col 2i, its ACT partial in col 2i+1,
        # so each row tile's partials are contiguous and need no combine op.
        sums = stats.tile([P, 2 * N_PARTS], mybir.dt.float32)
        dummy = stats.tile([P, CHUNK * 3 // 8], mybir.dt.float32)
        ones = stats.tile([P, 1], mybir.dt.float32)
        nc.vector.memset(ones[:], 1.0)

        # (row_tile, col_offset, width) per chunk, linear order.
        chunks = []
        for r in range(N_RT - 1):
            chunks.append((r, 0, CHUNK))
            chunks.append((r, CHUNK, CHUNK))
        chunks.append((N_RT - 1, 0, CHUNK))
        col = CHUNK
        for w in TAIL_CHUNKS:
            chunks.append((N_RT - 1, col, w))
            col += w

        for i, (r, c0, w) in enumerate(chunks):
            t = data.tile([P, w], mybir.dt.float32, tag="t")
            nc.sync.dma_start(t[:], B[r * P : (r + 1) * P, c0 : c0 + w])
            dve_cols = TAIL_DVE_COLS.get(i, (w * 5) // 8)
            nc.vector.reduce_sum(
                sums[:, 2 * i : 2 * i + 1], t[:, :dve_cols], axis=mybir.AxisListType.X
            )
            nc.scalar.activation(
                dummy[:, : w - dve_cols],
                t[:, dve_cols:w],
                mybir.ActivationFunctionType.Copy,
                accum_out=sums[:, 2 * i + 1 : 2 * i + 2],
            )

        # Per-row-tile sums: full tiles have 4 contiguous partials, tile 7
        # has 8 (one full chunk + three tail chunks).
        n_t7 = 2 * (1 + len(TAIL_CHUNKS))
        rowsums = stats.tile([P, N_RT], mybir.dt.float32)
        nc.vector.reduce_sum(
            rowsums[:, : N_RT - 1],
            sums[:, : 2 * N_PARTS - n_t7].rearrange("p (r c) -> p r c", c=4),
            axis=mybir.AxisListType.X,
        )
        nc.vector.reduce_sum(
            rowsums[:, N_RT - 1 : N_RT],
            sums[:, 2 * N_PARTS - n_t7 :],
            axis=mybir.AxisListType.X,
        )

        # hinge = max(rowsum - 1, 0); per-partition sum; partition-collapse.
        hinges = stats.tile([P, N_RT], mybir.dt.float32)
        nc.vector.tensor_scalar(
            hinges[:],
            rowsums[:],
            -1.0,
            0.0,
            op0=mybir.AluOpType.add,
            op1=mybir.AluOpType.max,
        )
        hsum = stats.tile([P, 1], mybir.dt.float32)
        nc.vector.reduce_sum(hsum[:], hinges[:], axis=mybir.AxisListType.X)

        acc = psum.tile([1, 1], mybir.dt.float32)
        nc.tensor.matmul(acc[:], ones[:], hsum[:], start=True, stop=True)
        res = stats.tile([1, 1], mybir.dt.float32)
        nc.scalar.copy(res[:], acc[:])
        nc.sync.dma_start(out[:], res[:])

    nc.compile()
    return nc


def _run(B: np.ndarray, trace: bool = False):
    global _PROGRAM
    if _PROGRAM is None:
        _PROGRAM = _build_program()
    in_maps = [
        {"B": B[i * SHARD_ROWS : (i + 1) * SHARD_ROWS]} for i in range(N_CORES)
    ]
    res = run_bass_kernel_spmd(_PROGRAM, in_maps, list(range(N_CORES)), trace=trace)
    total = float(sum(np.float64(r["out"][0, 0]) for r in res.results))
    value = np.asarray(np.float32(PENALTY_B * total / ROWS))
    return value, res


def kernel(B: np.ndarray) -> np.ndarray:
    B = np.ascontiguousarray(np.asarray(B, dtype=np.float32))
    assert B.shape == (ROWS, COLS), B.shape
    value, _ = _run(B, trace=False)
    return value
